# revision 4
# baseline (speedup 1.0000x reference)
"""DynamicVoxelizer Bass/Trainium2 kernel.

Contract: kernel(**inputs) takes the FULL inputs (points: [8, 1_000_000, 3]
float32), distributes across 8 NeuronCores (one batch per core), runs a Bass
kernel per core, and returns the FULL outputs matching reference():
  (out_points f32 [8,1M,3], coords_zyx i32 [8,1M,3], point_idxes i32 [8,1M],
   point_offsets f32 [8,1M,3], valid bool [8,1M])

Numerics: XLA compiles the reference's (p - min) / 0.2 into (p - min) * 5.0
(verified bit-exact on both the CPU and neuron backends), so the device
computes q = RN(5 * RN(p - min)) and floor(q) via an RNE cast + compare
fix-up (the hardware f32->i32 conversion rounds to nearest-even).
"""

import numpy as np

import concourse.bass as bass
import concourse.mybir as mybir
import concourse.tile as tile
from concourse.bass_utils import run_bass_kernel_spmd

AOT = mybir.AluOpType
AFT = mybir.ActivationFunctionType
F32 = mybir.dt.float32
I32 = mybir.dt.int32
I16 = mybir.dt.int16
U8 = mybir.dt.uint8

# Problem constants (hardcoded; kernel.py must be self-contained)
B = 8                      # batch == number of cores
NPTS = 1_000_000           # points per batch
P = 128                    # SBUF partitions
NPER = 7840                # points per partition (padded): 128*7840 = 1_003_520
NPAD = P * NPER            # padded points per core
FPER = NPER * 3            # floats per partition = 23520
CH_N = 980                 # points per partition per chunk
CH_F = CH_N * 3            # 2940 floats
NCHUNKS = NPER // CH_N     # 8
PAD_VAL = 1.0e9            # pad points are far out of range -> invalid

MINS = (-51.2, -51.2, -3.0)
GRIDF = (512.0, 512.0, 30.0)
# largest fp32 strictly below the grid bound: q < g  <=>  q <= gprime
GPRIME = tuple(float(np.nextafter(np.float32(g), np.float32(0))) for g in GRIDF)

_cached = {}


def _split_excess_waits(nc, limit=1):
    """walrus on this container rejects instructions with more than `limit`
    sync-wait conditions; split the excess into standalone event-sem waits."""
    for fn in nc.m.functions:
        for bb in fn.blocks:
            il = bb.instructions
            new = []
            for inst in il:
                si = getattr(inst, "sync_info", None)
                if si is not None and si.on_wait and len(si.on_wait) > limit:
                    waits = list(si.on_wait)
                    excess, keep = waits[:-limit], waits[-limit:]
                    for j, w in enumerate(excess):
                        ev = mybir.InstEventSemaphore(
                            name=f"{inst.name}-xw{j}", ins=[], outs=[],
                            sync_info=mybir.SyncInfo(on_wait=[w], on_update=[]),
                        )
                        ev.engine = inst.engine
                        new.append(ev)
                    inst.sync_info = mybir.SyncInfo(
                        on_wait=keep, on_update=list(si.on_update))
                new.append(inst)
            il[:] = new


def _build_kernel():
    nc = bass.Bass()
    # const APs for activation biases (Abs/Sign need AP biases)
    for v in sorted({-g for g in GPRIME}):
        t = nc.alloc_sbuf_tensor(f"const-f32-{v}", [P, 1], F32)
        nc.gpsimd.memset(t.ap(), v)
        nc.const_aps.aps[(F32, v)] = t.ap()
    nc.all_engine_barrier()

    pts = nc.dram_tensor("pts", [P, FPER], F32, kind="ExternalInput")
    o_pts = nc.dram_tensor("o_pts", [P, FPER], F32, kind="ExternalOutput")
    o_coords = nc.dram_tensor("o_coords", [P, FPER], I16, kind="ExternalOutput")
    o_off = nc.dram_tensor("o_off", [P, FPER], F32, kind="ExternalOutput")
    o_idx = nc.dram_tensor("o_idx", [P, NPER], I32, kind="ExternalOutput")
    o_valid = nc.dram_tensor("o_valid", [P, NPER], U8, kind="ExternalOutput")

    with tile.TileContext(nc) as tc:
        with tc.tile_pool(name="io", bufs=2) as iop, \
             tc.tile_pool(name="scr", bufs=1) as scr:
            for ch in range(NCHUNKS):
                f0_ = ch * CH_F     # float column offset
                n0_ = ch * CH_N     # point column offset

                pt = iop.tile([P, CH_F], F32, tag="pts")
                nc.sync.dma_start(pt[:], pts[:, f0_:f0_ + CH_F])
                pt3 = pt[:].rearrange("p (n c) -> p n c", c=3)

                # t3 = p - min (per component, strided)  [ACT]
                t3 = scr.tile([P, CH_F], F32, tag="t3")
                t33 = t3[:].rearrange("p (n c) -> p n c", c=3)
                for c in range(3):
                    nc.scalar.activation(t33[:, :, c], pt3[:, :, c],
                                         AFT.Copy, bias=-MINS[c], scale=1.0)
                # q0 = RN(5*t)  [ACT]
                q0 = scr.tile([P, CH_F], F32, tag="q0")
                nc.scalar.activation(q0[:], t3[:], AFT.Copy, bias=0.0, scale=5.0)

                # floor(q0): RNE cast + fixup   [ACT casts, DVE fix]
                i0 = scr.tile([P, CH_F], I32, tag="C")
                nc.scalar.copy(i0[:], q0[:])
                fl = scr.tile([P, CH_F], F32, tag="A")
                nc.scalar.copy(fl[:], i0[:])
                h = scr.tile([P, CH_F], F32, tag="D")
                nc.vector.scalar_tensor_tensor(
                    h[:], q0[:], 0.0, fl[:], AOT.add, AOT.is_lt)
                cf = scr.tile([P, CH_F], F32, tag="B")
                nc.vector.scalar_tensor_tensor(
                    cf[:], h[:], -1.0, fl[:], AOT.mult, AOT.add)
                cf3 = cf[:].rearrange("p (n c) -> p n c", c=3)

                # validity: w = |2*q0 - g'|; valid_c <=> w <= g'
                # sign trick: sg = Sign(w - g') ; valid <=> max(sg) <= 0
                w3 = scr.tile([P, CH_F], F32, tag="E")
                w33 = w3[:].rearrange("p (n c) -> p n c", c=3)
                q03 = q0[:].rearrange("p (n c) -> p n c", c=3)
                for c in range(3):
                    nc.scalar.activation(w33[:, :, c], q03[:, :, c],
                                         AFT.Abs, bias=-GPRIME[c], scale=2.0)
                sg = scr.tile([P, CH_F], F32, tag="E2")
                sg3 = sg[:].rearrange("p (n c) -> p n c", c=3)
                for c in range(3):
                    nc.scalar.activation(sg3[:, :, c], w33[:, :, c],
                                         AFT.Sign, bias=-GPRIME[c], scale=1.0)
                v01 = scr.tile([P, CH_N], F32, tag="v01")
                nc.vector.tensor_tensor(v01[:], sg3[:, :, 0], sg3[:, :, 1], AOT.max)
                mx = scr.tile([P, CH_N], F32, tag="mx")
                nc.vector.tensor_tensor(mx[:], v01[:], sg3[:, :, 2], AOT.max)
                valid = iop.tile([P, CH_N], U8, tag="valid")
                nc.vector.tensor_scalar(valid[:], mx[:], 0.0, None, AOT.is_le)

                # coords (z,y,x), -1 fill  [GP memset + DVE cp]
                cmem = iop.tile([P, CH_F], I16, tag="cmem")
                nc.gpsimd.memset(cmem[:], -1)
                cm3 = cmem[:].rearrange("p (n c) -> p n c", c=3)
                for c in range(3):
                    nc.vector.copy_predicated(cm3[:, :, 2 - c], valid[:],
                                              cf3[:, :, c])

                # offsets: off = ((cf*-0.2 + t3) - 0.1) * valid   [DVE]
                offm = iop.tile([P, CH_F], F32, tag="offm")
                nc.vector.scalar_tensor_tensor(
                    offm[:], cf[:], -0.2, t3[:], AOT.mult, AOT.add)
                vbc = valid[:].unsqueeze(2).broadcast_to([P, CH_N, 3])
                off3 = offm[:].rearrange("p (n c) -> p n c", c=3)
                nc.vector.scalar_tensor_tensor(
                    off3, off3, 0.1, vbc, AOT.subtract, AOT.mult)

                # out_points: p * valid  [GP, strided in-place]
                for c in range(3):
                    nc.gpsimd.tensor_tensor(pt3[:, :, c], pt3[:, :, c],
                                            valid[:], AOT.mult)

                # point idxes: iota, -1 fill, cp  [GP + DVE]
                iot = scr.tile([P, CH_N], I32, tag="iot")
                nc.gpsimd.iota(iot[:], [[1, CH_N]], base=n0_,
                               channel_multiplier=NPER)
                imem = iop.tile([P, CH_N], I32, tag="imem")
                nc.gpsimd.memset(imem[:], -1)
                nc.vector.copy_predicated(imem[:], valid[:], iot[:])

                # stores
                nc.sync.dma_start(o_pts[:, f0_:f0_ + CH_F], pt[:])
                nc.sync.dma_start(o_coords[:, f0_:f0_ + CH_F], cmem[:])
                nc.sync.dma_start(o_off[:, f0_:f0_ + CH_F], offm[:])
                nc.sync.dma_start(o_idx[:, n0_:n0_ + CH_N], imem[:])
                nc.sync.dma_start(o_valid[:, n0_:n0_ + CH_N], valid[:])

    _split_excess_waits(nc, limit=1)
    return nc


def _get_nc():
    if "nc" not in _cached:
        _cached["nc"] = _build_kernel()
    return _cached["nc"]


def _make_in_maps(points):
    in_maps = []
    for b in range(B):
        flat = np.full(NPAD * 3, PAD_VAL, dtype=np.float32)
        flat[: NPTS * 3] = points[b].reshape(-1)
        in_maps.append({"pts": flat.reshape(P, FPER)})
    return in_maps


def kernel(points: np.ndarray):
    points = np.asarray(points)
    assert points.shape == (B, NPTS, 3) and points.dtype == np.float32

    nc = _get_nc()
    res = run_bass_kernel_spmd(nc, _make_in_maps(points),
                               core_ids=list(range(B)))

    out_points = np.empty((B, NPTS, 3), dtype=np.float32)
    coords = np.empty((B, NPTS, 3), dtype=np.int32)
    idxes = np.empty((B, NPTS), dtype=np.int32)
    offsets = np.empty((B, NPTS, 3), dtype=np.float32)
    valid = np.empty((B, NPTS), dtype=bool)
    for b in range(B):
        r = res.results[b]
        out_points[b] = r["o_pts"].reshape(-1)[: NPTS * 3].reshape(NPTS, 3)
        coords[b] = (r["o_coords"].astype(np.int32)
                     .reshape(-1)[: NPTS * 3].reshape(NPTS, 3))
        offsets[b] = r["o_off"].reshape(-1)[: NPTS * 3].reshape(NPTS, 3)
        idxes[b] = r["o_idx"].reshape(-1)[: NPTS]
        valid[b] = r["o_valid"].reshape(-1)[: NPTS].astype(bool)
    return out_points, coords, idxes, offsets, valid


# revision 8
# speedup vs baseline: 3.2611x; 3.2611x over previous
"""DynamicVoxelizer Bass/Trainium2 kernel.

Contract: kernel(**inputs) takes the FULL inputs (points: [8, 1_000_000, 3]
float32), distributes across 8 NeuronCores (one batch per core), runs a Bass
kernel per core, and returns the FULL outputs matching reference():
  (out_points f32 [8,1M,3], coords_zyx i32 [8,1M,3], point_idxes i32 [8,1M],
   point_offsets f32 [8,1M,3], valid bool [8,1M])

Numerics: XLA compiles the reference's (p - min) / 0.2 into (p - min) * 5.0
(verified bit-exact on both the CPU and neuron backends), so the device
computes q = RN(5 * RN(p - min)) and floor(q) via an RNE cast + compare
fix-up (the hardware f32->i32 conversion rounds to nearest-even).
"""

import numpy as np

import concourse.bass as bass
import concourse.mybir as mybir
import concourse.tile as tile
from concourse.bass_utils import run_bass_kernel_spmd

AOT = mybir.AluOpType
AFT = mybir.ActivationFunctionType
F32 = mybir.dt.float32
I32 = mybir.dt.int32
I16 = mybir.dt.int16
U8 = mybir.dt.uint8

# Problem constants (hardcoded; kernel.py must be self-contained)
B = 8                      # batch == number of cores
NPTS = 1_000_000           # points per batch
P = 128                    # SBUF partitions
NPER = 7840                # points per partition (padded): 128*7840 = 1_003_520
NPAD = P * NPER            # padded points per core
FPER = NPER * 3            # floats per partition = 23520
CH_N = 980                 # points per partition per chunk
CH_F = CH_N * 3            # 2940 floats
NCHUNKS = NPER // CH_N     # 8
PAD_VAL = 1.0e9            # pad points are far out of range -> invalid

MINS = (-51.2, -51.2, -3.0)
GRIDF = (512.0, 512.0, 30.0)
# largest fp32 strictly below the grid bound: q < g  <=>  q <= gprime
GPRIME = tuple(float(np.nextafter(np.float32(g), np.float32(0))) for g in GRIDF)

_cached = {}


def _split_excess_waits(nc, limit=1):
    """walrus on this container rejects instructions with more than `limit`
    sync-wait conditions; split the excess into standalone event-sem waits."""
    for fn in nc.m.functions:
        for bb in fn.blocks:
            il = bb.instructions
            new = []
            for inst in il:
                si = getattr(inst, "sync_info", None)
                if si is not None and si.on_wait and len(si.on_wait) > limit:
                    waits = list(si.on_wait)
                    excess, keep = waits[:-limit], waits[-limit:]
                    for j, w in enumerate(excess):
                        ev = mybir.InstEventSemaphore(
                            name=f"{inst.name}-xw{j}", ins=[], outs=[],
                            sync_info=mybir.SyncInfo(on_wait=[w], on_update=[]),
                        )
                        ev.engine = inst.engine
                        new.append(ev)
                    inst.sync_info = mybir.SyncInfo(
                        on_wait=keep, on_update=list(si.on_update))
                new.append(inst)
            il[:] = new


def _build_kernel():
    nc = bass.Bass()
    # const APs for activation biases (Abs/Sign need AP biases)
    for v in sorted({-g for g in GPRIME}):
        t = nc.alloc_sbuf_tensor(f"const-f32-{v}", [P, 1], F32)
        nc.gpsimd.memset(t.ap(), v)
        nc.const_aps.aps[(F32, v)] = t.ap()
    nc.all_engine_barrier()

    pts = nc.dram_tensor("pts", [P, FPER], F32, kind="ExternalInput")
    o_pts = nc.dram_tensor("o_pts", [P, FPER], F32, kind="ExternalOutput")
    o_coords = nc.dram_tensor("o_coords", [P, FPER], I16, kind="ExternalOutput")
    o_off = nc.dram_tensor("o_off", [P, FPER], F32, kind="ExternalOutput")
    o_idx = nc.dram_tensor("o_idx", [P, NPER], I32, kind="ExternalOutput")
    o_valid = nc.dram_tensor("o_valid", [P, NPER], U8, kind="ExternalOutput")

    with tile.TileContext(nc) as tc:
        with tc.tile_pool(name="io", bufs=2) as iop, \
             tc.tile_pool(name="scr", bufs=1) as scr:
            for ch in range(NCHUNKS):
                f0_ = ch * CH_F     # float column offset
                n0_ = ch * CH_N     # point column offset

                pt = iop.tile([P, CH_F], F32, tag="pts")
                nc.sync.dma_start(pt[:], pts[:, f0_:f0_ + CH_F])
                pt3 = pt[:].rearrange("p (n c) -> p n c", c=3)

                # t3 = p - min (per component, strided)  [ACT]
                t3 = scr.tile([P, CH_F], F32, tag="t3")
                t33 = t3[:].rearrange("p (n c) -> p n c", c=3)
                for c in range(3):
                    nc.scalar.activation(t33[:, :, c], pt3[:, :, c],
                                         AFT.Copy, bias=-MINS[c], scale=1.0)
                # q0 = RN(5*t)  [ACT]
                q0 = scr.tile([P, CH_F], F32, tag="q0")
                nc.scalar.activation(q0[:], t3[:], AFT.Copy, bias=0.0, scale=5.0)

                # floor(q0): RNE cast + fixup   [ACT casts, DVE fix]
                i0 = scr.tile([P, CH_F], I32, tag="C")
                nc.scalar.copy(i0[:], q0[:])
                flp = scr.tile([P, CH_F], F32, tag="A")   # rne(q0) + 1
                nc.scalar.activation(flp[:], i0[:], AFT.Copy,
                                     bias=1.0, scale=1.0)
                h = scr.tile([P, CH_F], F32, tag="D")
                nc.vector.scalar_tensor_tensor(
                    h[:], q0[:], 1.0, flp[:], AOT.add, AOT.is_lt)
                # cfp1 = floor(q0) + 1 = flp - h
                cfp1 = scr.tile([P, CH_F], F32, tag="B")
                nc.vector.scalar_tensor_tensor(
                    cfp1[:], h[:], -1.0, flp[:], AOT.mult, AOT.add)
                cf31 = cfp1[:].rearrange("p (n c) -> p n c", c=3)

                # validity: w = |2*q0 - g'|; valid_c <=> w <= g'
                # sign trick: sg = Sign(w - g') ; valid <=> max(sg) <= 0
                w3 = scr.tile([P, CH_F], F32, tag="E")
                w33 = w3[:].rearrange("p (n c) -> p n c", c=3)
                q03 = q0[:].rearrange("p (n c) -> p n c", c=3)
                for c in range(3):
                    nc.scalar.activation(w33[:, :, c], q03[:, :, c],
                                         AFT.Abs, bias=-GPRIME[c], scale=2.0)
                sg = scr.tile([P, CH_F], F32, tag="E2")
                sg3 = sg[:].rearrange("p (n c) -> p n c", c=3)
                for c in range(3):
                    nc.scalar.activation(sg3[:, :, c], w33[:, :, c],
                                         AFT.Sign, bias=-GPRIME[c], scale=1.0)
                v01 = scr.tile([P, CH_N], F32, tag="v01")
                nc.vector.tensor_tensor(v01[:], sg3[:, :, 0], sg3[:, :, 1], AOT.max)
                mx = scr.tile([P, CH_N], F32, tag="mx")
                nc.vector.tensor_tensor(mx[:], v01[:], sg3[:, :, 2], AOT.max)
                valid = iop.tile([P, CH_N], U8, tag="valid")
                nc.vector.tensor_scalar(valid[:], mx[:], 0.0, None, AOT.is_le)

                # coords+1 = valid * (floor+1), u16 out; host subtracts 1
                # and reverses xyz->zyx. Invalid lanes -> 0 -> host -1.
                vbc = valid[:].unsqueeze(2).broadcast_to([P, CH_N, 3])
                cmem = iop.tile([P, CH_F], I16, tag="cmem")
                cm3 = cmem[:].rearrange("p (n c) -> p n c", c=3)
                nc.vector.tensor_tensor(cm3, cf31, vbc, AOT.mult)

                # offsets: off = ((cf*-0.2 + t3) - 0.1) * valid   [DVE]
                offm = iop.tile([P, CH_F], F32, tag="offm")
                nc.vector.scalar_tensor_tensor(
                    offm[:], cfp1[:], -0.2, t3[:], AOT.mult, AOT.add)
                off3 = offm[:].rearrange("p (n c) -> p n c", c=3)
                nc.vector.scalar_tensor_tensor(
                    off3, off3, 0.1, vbc, AOT.add, AOT.mult)

                # out_points: p * valid  [GP, interleaved in-place]
                nc.gpsimd.tensor_tensor(pt3, pt3, vbc, AOT.mult)

                # point idxes + 1 = valid * (iota+1); host subtracts 1
                iot = scr.tile([P, CH_N], I32, tag="iot")
                nc.gpsimd.iota(iot[:], [[1, CH_N]], base=n0_ + 1,
                               channel_multiplier=NPER)
                imem = iop.tile([P, CH_N], I32, tag="imem")
                nc.vector.tensor_tensor(imem[:], iot[:], valid[:], AOT.mult)

                # stores
                nc.sync.dma_start(o_pts[:, f0_:f0_ + CH_F], pt[:])
                nc.sync.dma_start(o_coords[:, f0_:f0_ + CH_F], cmem[:])
                nc.sync.dma_start(o_off[:, f0_:f0_ + CH_F], offm[:])
                nc.sync.dma_start(o_idx[:, n0_:n0_ + CH_N], imem[:])
                nc.sync.dma_start(o_valid[:, n0_:n0_ + CH_N], valid[:])

    _split_excess_waits(nc, limit=1)
    return nc


def _get_nc():
    if "nc" not in _cached:
        _cached["nc"] = _build_kernel()
    return _cached["nc"]


def _make_in_maps(points):
    in_maps = []
    for b in range(B):
        flat = np.full(NPAD * 3, PAD_VAL, dtype=np.float32)
        flat[: NPTS * 3] = points[b].reshape(-1)
        in_maps.append({"pts": flat.reshape(P, FPER)})
    return in_maps


def kernel(points: np.ndarray):
    points = np.asarray(points)
    assert points.shape == (B, NPTS, 3) and points.dtype == np.float32

    nc = _get_nc()
    res = run_bass_kernel_spmd(nc, _make_in_maps(points),
                               core_ids=list(range(B)))

    out_points = np.empty((B, NPTS, 3), dtype=np.float32)
    coords = np.empty((B, NPTS, 3), dtype=np.int32)
    idxes = np.empty((B, NPTS), dtype=np.int32)
    offsets = np.empty((B, NPTS, 3), dtype=np.float32)
    valid = np.empty((B, NPTS), dtype=bool)
    for b in range(B):
        r = res.results[b]
        out_points[b] = r["o_pts"].reshape(-1)[: NPTS * 3].reshape(NPTS, 3)
        coords[b] = (r["o_coords"].view(np.uint16).astype(np.int32)
                     .reshape(-1)[: NPTS * 3].reshape(NPTS, 3)[:, ::-1] - 1)
        offsets[b] = r["o_off"].reshape(-1)[: NPTS * 3].reshape(NPTS, 3)
        idxes[b] = r["o_idx"].reshape(-1)[: NPTS] - 1
        valid[b] = r["o_valid"].reshape(-1)[: NPTS].astype(bool)
    return out_points, coords, idxes, offsets, valid
